# revision 77
# baseline (speedup 1.0000x reference)
"""Trainium2 8-core tensor-parallel attention kernel (Bass/Tile), v4.

Sharding: heads tensor-parallel across 8 cores (2 heads/core) for
QKV + attention; output projection is column-sharded (each core owns
256 output channels) fed by per-chunk AllGathers of the pre-projection
attention outputs.

v19 over v3 (425.7us -> 388-400us on HW, collective-draw dependent):
  - partition-contiguous DRAM layouts for weights/x/mask so loads run
    at line rate as a few big DMAs (granular only where latency-
    critical: wq quarters + x-chunk-0 eighths at the head)
  - DMAs split across both HWDGE rings (sync: x + ag reads + out
    writes; scalar: weights/consts at head, ag_in writes, final agl7
    reads). Out writes deliberately NOT on the scalar ring: a stalled
    DMA at the scalar FIFO head would block the exp stream.
  - chunks 6+7 keep running probs-sums on the vector engine so their
    deferred normalizes (and AllGather triggers) fire right at their
    attention's end with no tree-reduction stall
  - NO O-proj woven before it7: the collective subsystem's startup
    varies 60-150us run-to-run, and an early-woven O-proj whose
    AllGather is late head-of-line-stalls the PE FIFO for tens of us
    (catastrophic +70us runs). oproj(0,1) weave into attn(7); oproj
    (2..6) run in the tail as ~48us of cover for AG(7)'s serial mesh
    cycle (~20-44us, barrier-bound, skew-variable), with reads staged
    two chunks ahead through the 4-buffer agl pool; a short dummy
    chain keeps the PE clock ramped; oproj(7) finishes.
  - warm-up collective pinned to priority 0 (no data consumers -> the
    Tile scheduler would otherwise emit it arbitrarily late, wedging
    the whole AllGather pipeline behind it)

Self-contained: hardcodes B=2, S=2048, DIM=2048, NH=16, HD=128.
"""
import math

import numpy as np

B, S_FULL, DIM, NH = 2, 2048, 2048, 16
HD = 128
N_CORES = 8
HPC = NH // N_CORES          # heads per core (2)
OC = HPC * HD                # q/k/v channels per core (256)
OCD = DIM // N_CORES         # output channels per core (256)
DT = DIM // 128              # dim tiles (16)
SC_W = 512                   # schunk width (cols of flattened seq)

_CACHE = {}


def _build(S):
    """Build the 8-core SPMD Bass graph for sequence length S (B=2 fixed)."""
    import concourse.bass as bass
    import concourse.mybir as mybir
    import concourse.tile as tile
    from concourse import bacc

    fp32 = mybir.dt.float32
    bf16 = mybir.dt.bfloat16
    Exp = mybir.ActivationFunctionType.Exp
    Copy = mybir.ActivationFunctionType.Copy
    BYPASS = mybir.AluOpType.bypass

    FLAT = B * S                 # flattened rows (4096)
    NSC = FLAT // SC_W           # schunks / chunks (8)
    NQT = S // 128               # k-tiles per batch (16)
    SCALE = 1.0 / math.sqrt(HD)
    SWAP_MASK = [j + 1 - 2 * (j % 2) for j in range(32)]
    rg = [list(range(N_CORES))]

    nc = bacc.Bacc("TRN2", target_bir_lowering=False, debug=False,
                   num_devices=N_CORES)

    # ---- external parameters (partition-contiguous layouts) ----
    xt_d = nc.declare_dram_parameter("xt", [128, NSC, DT, SC_W], bf16,
                                     isOutput=False)
    wqt_d = nc.declare_dram_parameter("wqt", [128, DT, OC], bf16, isOutput=False)
    wkt_d = nc.declare_dram_parameter("wkt", [128, DT, OC], bf16, isOutput=False)
    wvt_d = nc.declare_dram_parameter("wvt", [128, DT, OC], bf16, isOutput=False)
    wot_d = nc.declare_dram_parameter("wotc", [128, DT, OCD], bf16, isOutput=False)
    cos_d = nc.declare_dram_parameter("cos_t", [HD, S], bf16, isOutput=False)
    sin_d = nc.declare_dram_parameter("sin_t", [HD, S], bf16, isOutput=False)
    mdg_d = nc.declare_dram_parameter("mask_diag", [128, NQT, 128], bf16,
                                      isOutput=False)
    on2_d = nc.declare_dram_parameter("ones128", [128, 128], bf16, isOutput=False)
    out_d = nc.declare_dram_parameter("outT", [OCD, FLAT], bf16, isOutput=True)

    # ---- internal DRAM (collective staging) ----
    LAST = NSC - 1
    ag_in_d = [nc.dram_tensor(f"ag_in{p}", [HPC, 128, SC_W], bf16)
               for p in range(NSC)]
    ag_out_d = [nc.dram_tensor(f"ag_out{p}", [DT, 128, SC_W], bf16,
                               addr_space="Shared") for p in range(NSC)]
    warm_in_d = nc.dram_tensor("warm_in", [8, 16], bf16)
    warm_out_d = nc.dram_tensor("warm_out", [64, 16], bf16, addr_space="Shared")

    from contextlib import ExitStack
    with tile.TileContext(nc) as tc:
        with ExitStack() as _stk:
            cpool = _stk.enter_context(tc.tile_pool(name="consts", bufs=1))
            wpool = _stk.enter_context(tc.tile_pool(name="wqkv", bufs=1))
            xpool = _stk.enter_context(tc.tile_pool(name="xT", bufs=2))
            qkpool = _stk.enter_context(tc.tile_pool(name="qk_sb", bufs=1))
            vpool = _stk.enter_context(tc.tile_pool(name="vbf", bufs=1))
            spool = _stk.enter_context(tc.tile_pool(name="p1tmp", bufs=2))
            tpool = _stk.enter_context(tc.tile_pool(name="t1tmp", bufs=2))
            ptpool = _stk.enter_context(tc.tile_pool(name="probsT", bufs=1))
            smpool = _stk.enter_context(tc.tile_pool(name="small", bufs=2))
            aglpool = _stk.enter_context(tc.tile_pool(name="agl", bufs=8))
            obpool = _stk.enter_context(tc.tile_pool(name="outsb", bufs=2))
            qkvps = _stk.enter_context(tc.tile_pool(name="qkvps", bufs=2, space="PSUM"))
            rotps = _stk.enter_context(tc.tile_pool(name="rotps", bufs=1, space="PSUM"))
            scps = _stk.enter_context(tc.tile_pool(name="scps", bufs=2, space="PSUM"))
            pops = _stk.enter_context(tc.tile_pool(name="pops", bufs=2, space="PSUM"))
            opps = _stk.enter_context(tc.tile_pool(name="opps", bufs=1, space="PSUM"))

            # warm-up collective: absorbs the cold-start mesh setup (~40-70us
            # on the first collective) behind the DMA head phase. Pinned to
            # priority 0 — it has no data consumers, so the Tile scheduler
            # would otherwise emit it arbitrarily late, stalling the whole
            # AllGather pipeline behind it.
            with tc.high_priority():
                nc.gpsimd.collective_compute(
                    "AllGather", BYPASS, replica_groups=rg,
                    ins=[warm_in_d[:]], outs=[warm_out_d[:]])

            # ---- head loads ----
            # scalar ring: weights + consts, ordered by first use
            w_sb = {}
            for nm, d in (("q", wqt_d), ("k", wkt_d), ("v", wvt_d)):
                w_sb[nm] = wpool.tile([128, DT, OC], bf16, tag=f"w{nm}",
                                      name=f"w{nm}")
            cos_sb = cpool.tile([HD, S], bf16)
            sin_sb = cpool.tile([HD, S], bf16)
            mdg_sb = cpool.tile([128, NQT, 128], bf16)
            on2_sb = cpool.tile([128, 128], bf16)
            wot_sb = cpool.tile([128, DT, OCD], bf16)

            # wq in quarters so the first q-chain MM starts ~1.5us in; the
            # later tensors stream in bulk behind it
            for q4 in range(4):
                nc.scalar.dma_start(w_sb["q"][:, q4 * 4:(q4 + 1) * 4, :],
                                    wqt_d[:, q4 * 4:(q4 + 1) * 4, :])
            nc.scalar.dma_start(w_sb["k"][:, 0:DT // 2, :],
                                wkt_d[:, 0:DT // 2, :])
            nc.scalar.dma_start(w_sb["k"][:, DT // 2:DT, :],
                                wkt_d[:, DT // 2:DT, :])
            nc.scalar.dma_start(cos_sb[:], cos_d[:])
            nc.scalar.dma_start(sin_sb[:], sin_d[:])
            nc.scalar.dma_start(w_sb["v"][:], wvt_d[:])
            nc.scalar.dma_start(on2_sb[:], on2_d[:])
            nc.scalar.dma_start(mdg_sb[:], mdg_d[:])
            nc.scalar.dma_start(wot_sb[:], wot_d[:])

            # sync ring: x chunk 0 in eighths for low first-MM latency
            xts0 = xpool.tile([128, DT, SC_W], bf16, tag="xt", name="xt0")
            for e8 in range(8):
                nc.sync.dma_start(xts0[:, e8 * 2:(e8 + 1) * 2, :],
                                  xt_d[:, 0, e8 * 2:(e8 + 1) * 2, :])

            # PE warm-up while the head DMAs stream: ramps the HAM clock
            # gate (~3.4us of sustained activity) so the first real chains
            # run at full rate instead of 1.2GHz. Reads an uninitialized
            # tile on purpose — no writer means no wait, so the chain
            # starts the moment the PE queue opens; the accumulator is
            # never consumed. Priority-pinned (no data consumers, the
            # scheduler would otherwise emit it arbitrarily late).
            with tc.high_priority():
                wrm = obpool.tile([128, SC_W], bf16, tag="ob", name="wrm")
                nc.vector.memset(wrm[:], 0)
                wps = opps.tile([128, SC_W], fp32, tag="op", name="wps")
                for i in range(16):
                    nc.tensor.matmul(wps[:], wrm[:, 0:128], wrm[:],
                                     start=(i == 0), stop=(i == 15))

            # persistent SBUF tensors
            qTa, kTa, vbfa = {}, {}, {}
            for bb in range(B):
                for h in range(HPC):
                    qTa[(bb, h)] = qkpool.tile([128, S], bf16, tag=f"qT{bb}{h}",
                                               name=f"qT{bb}{h}")
                    kTa[(bb, h)] = qkpool.tile([128, S], bf16, tag=f"kT{bb}{h}",
                                               name=f"kT{bb}{h}")
                    vbfa[(bb, h)] = vpool.tile([128, NQT, HD], bf16,
                                               tag=f"v{bb}{h}", name=f"v{bb}{h}")

            # state carried between loop iterations for deferred normalize
            pending = {}   # chunk p -> (qg, b, {h: (pt_buf, po_ps)})
            # pending work quanta (generators), pumped between attention
            # kt-steps to fill the PE's exp-wait micro-gaps
            op_queue = []

            def pump(n):
                k = 0
                while op_queue and k < n:
                    try:
                        next(op_queue[0])
                        k += 1
                    except StopIteration:
                        op_queue.pop(0)

            def flush_ops():
                while op_queue:
                    try:
                        next(op_queue[0])
                    except StopIteration:
                        op_queue.pop(0)

            # ============ deferred normalize + AllGather ============
            def normalize_chunk(p):
                qg, b, hstate = pending.pop(p)
                for h in range(HPC):
                    sum_src, po_ps = hstate[h]
                    sb_ps = rotps.tile([128, SC_W], fp32, tag="rot",
                                       name="sb_ps")
                    nc.tensor.matmul(sb_ps[:], on2_sb[:], sum_src,
                                     start=True, stop=True)
                    rbc = smpool.tile([128, SC_W], fp32, tag="rbc", name="rbc")
                    nc.vector.reciprocal_approx_fast(rbc[:], sb_ps[:])
                    ob = smpool.tile([128, SC_W], bf16, tag="obuf", name="ob")
                    nc.vector.tensor_mul(ob[:], po_ps[:], rbc[:])
                    nc.scalar.dma_start(ag_in_d[p][h, :, :], ob[:])
                nc.gpsimd.collective_compute(
                    "AllGather", BYPASS, replica_groups=rg,
                    ins=[ag_in_d[p][:]], outs=[ag_out_d[p][:]])

            # ============ per-chunk attention (h sections only) ============
            def attn_chunk(qg, b, pump_n=2, inc_sum=False):
                kmax = qg * 4 + 3
                K = kmax + 1
                p = b * 4 + qg
                hstate = {}
                for h in range(HPC):
                    po_ps = pops.tile([128, SC_W], fp32, tag="po", name=f"po{h}")
                    ptb = ptpool.tile([128, NQT, SC_W], bf16, tag=f"pTb{h}",
                                      name=f"pTb{h}")
                    if inc_sum:
                        # running probs-sum: ready right after the last exp, so
                        # the deferred normalize can be emitted before the next
                        # attention section reuses this chunk's buffers
                        sum_sb = smpool.tile([128, SC_W], bf16, tag="isum",
                                             name=f"isum{h}")
                    for kt in range(K):
                        qlo = max(0, kt - qg * 4) * 128
                        n = SC_W - qlo
                        sp = scps.tile([128, SC_W], fp32, tag="sc", name="sp")
                        nc.tensor.matmul(
                            sp[:, :n],
                            kTa[(b, h)][:, kt * 128:(kt + 1) * 128],
                            qTa[(b, h)][:, qg * SC_W + qlo:(qg + 1) * SC_W],
                            start=True, stop=True)
                        if kt >= qg * 4:
                            nc.vector.tensor_add(
                                sp[:, 0:128], sp[:, 0:128], mdg_sb[:, kt, :])
                        nc.scalar.activation(ptb[:, kt, qlo:SC_W], sp[:, :n], Exp)
                        if inc_sum:
                            # accumulate only the exp-written region; the
                            # masked-out cols of diagonal tiles are never
                            # read (no memsets needed)
                            if kt == 0:
                                nc.vector.tensor_copy(sum_sb[:], ptb[:, 0, :])
                            else:
                                nc.vector.tensor_add(
                                    sum_sb[:, qlo:], sum_sb[:, qlo:],
                                    ptb[:, kt, qlo:])
                        if kt >= 1:
                            kl = kt - 1
                            ql2 = max(0, kl - qg * 4) * 128
                            nc.tensor.matmul(
                                po_ps[:, ql2:SC_W], vbfa[(b, h)][:, kl, :],
                                ptb[:, kl, ql2:SC_W],
                                start=(kl == 0), stop=False)
                        pump(pump_n)
                    ql2 = max(0, kmax - qg * 4) * 128
                    nc.tensor.matmul(
                        po_ps[:, ql2:SC_W], vbfa[(b, h)][:, kmax, :],
                        ptb[:, kmax, ql2:SC_W], start=(kmax == 0), stop=True)
                    if inc_sum:
                        hstate[h] = (sum_sb[:], po_ps)
                    else:
                        # in-place bf16 tree reduction over the full-width
                        # tiles -> ptb[:,0,:], then fold in each diagonal
                        # tile's valid region (masked-out cols never read)
                        kk = qg * 4 + 1
                        while kk > 1:
                            m = kk // 2
                            nc.vector.tensor_add(ptb[:, 0:m, :], ptb[:, 0:m, :],
                                                 ptb[:, kk - m:kk, :])
                            kk -= m
                        for kt in range(qg * 4 + 1, kmax + 1):
                            qlo = (kt - qg * 4) * 128
                            nc.vector.tensor_add(
                                ptb[:, 0, qlo:], ptb[:, 0, qlo:],
                                ptb[:, kt, qlo:])
                        hstate[h] = (ptb[:, 0, :], po_ps)
                pending[p] = (qg, b, hstate)

            # ============ per-chunk O-projection (column-sharded) ============
            def agl_prefetch(p):
                """Issue the reads of chunk p's AllGathered attention
                outputs (4 x 512KB); returns the SBUF tiles."""
                agl = []
                for i in range(4):
                    t = aglpool.tile([128, 4, SC_W], bf16, tag="agl",
                                     name=f"agl{i}")
                    nc.sync.dma_start(
                        t[:], ag_out_d[p][i * 4:(i + 1) * 4, :, :]
                        .rearrange("t p w -> p t w"))
                    agl.append(t)
                return agl

            def oproj_emit(p, slab_order, moving, pump_gate=None):
                """Emit the two accumulation chains of chunk p's O-proj.
                slab_order gives the contraction order (any permutation of
                range(DT)); moving(e) returns the [128, SC_W] slab AP."""
                for dh in range(2):
                    # dh=1 borrows the rot pool's bank so the two half-chains
                    # don't serialize on a single PSUM drain
                    pool = opps if dh == 0 else rotps
                    op_ps = pool.tile([128, SC_W], fp32,
                                      tag="op" if dh == 0 else "rot",
                                      name="op_ps")
                    for j, e in enumerate(slab_order):
                        nc.tensor.matmul(
                            op_ps[:],
                            wot_sb[:, e, dh * 128:(dh + 1) * 128],
                            moving(e),
                            start=(j == 0), stop=(j == DT - 1))
                        yield
                    obt = obpool.tile([128, SC_W], bf16, tag="ob", name="obt")
                    if dh == 0:
                        nc.scalar.copy(obt[:], op_ps[:])
                    else:
                        nc.vector.tensor_copy(obt[:], op_ps[:])
                    # out writes ride the sync ring: a late O-proj chain must
                    # not head-of-line-block the scalar queue's exp stream
                    nc.sync.dma_start(
                        out_d[dh * 128:(dh + 1) * 128,
                              p * SC_W:(p + 1) * SC_W], obt[:])

            def oproj_gen(p, agl=None, fillers=6):
                """Generator: one yield per matmul so the caller can weave
                the chain between attention steps. Filler yields after the
                agl reads give the transfers time to land before the first
                matmul hits the PE queue."""
                if agl is None:
                    agl = agl_prefetch(p)
                    for _ in range(fillers):
                        yield
                yield from oproj_emit(p, range(DT),
                                      lambda e: agl[e // 4][:, e % 4, :])

            def oproj_chunk(p, agl=None):
                for _ in oproj_gen(p, agl=agl):
                    pass

            # ============ main fused loop ============
            # normalize target at iteration start = previous chunk
            # O-proj: nothing woven before it7 — the collective subsystem's
            # startup varies 60-150us run-to-run, and an early-woven O-proj
            # whose AllGather is late head-of-line-stalls the PE FIFO for
            # tens of us. Chunks 0+1 weave into attn(7) (their AllGathers
            # are done by then in every run); chunks 2..6 project in the
            # tail (~48us of cover for AG(7)'s 20-44us mesh cycle).

            carried_xts = None
            for it in range(NSC):
                sc = it                       # QKV chunk == iteration index
                b, qg = divmod(sc, 4)
                c0 = qg * SC_W               # column offset within batch
                xts = xts0 if sc == 0 else carried_xts

                def emit_rope_mm(t, h, til):
                    # pair-swap rotation on the vector engine (sign folded
                    # into the sin table host-side) - no PE or PSUM involved
                    rt = spool.tile([128, SC_W], bf16, tag="rt", name="rt")
                    nc.vector.stream_shuffle(rt[:], til[:], SWAP_MASK)
                    dst = (qTa if t == "q" else kTa)[(b, h)]
                    t1 = tpool.tile([128, SC_W], bf16, tag="t1", name="t1")
                    # vector, not gpsimd: the gpsimd queue must stay
                    # trigger-only (a backlogged collective trigger there
                    # would head-of-line-stall everything behind it)
                    nc.vector.tensor_mul(t1[:], til[:], cos_sb[:, c0:c0 + SC_W])
                    hat = spool.tile([128, SC_W], bf16, tag="hat", name="hat")
                    nc.vector.tensor_mul(hat[:], rt[:], sin_sb[:, c0:c0 + SC_W])
                    nc.vector.tensor_add(dst[:, c0:c0 + SC_W], hat[:], t1[:])

                def v_gen(vsc, vxts):
                    """Natural-layout V chains for schunk vsc (stationary =
                    x tile, both heads at once). No scalar inputs, so these
                    weave safely into exp-bound attention sections."""
                    vb, vqg = divmod(vsc, 4)
                    for vt in range(4):
                        psv = qkvps.tile([128, OC], fp32, tag="qkv",
                                         name="ps_v")
                        for dt in range(DT):
                            nc.tensor.matmul(
                                psv[:],
                                vxts[:, dt, vt * 128:(vt + 1) * 128],
                                w_sb["v"][:, dt, :],
                                start=(dt == 0), stop=(dt == DT - 1))
                            yield
                        for hh in range(HPC):
                            if hh == 0:
                                nc.scalar.copy(
                                    vbfa[(vb, hh)][:, vqg * 4 + vt, :],
                                    psv[:, hh * HD:(hh + 1) * HD])
                            else:
                                nc.vector.tensor_copy(
                                    vbfa[(vb, hh)][:, vqg * 4 + vt, :],
                                    psv[:, hh * HD:(hh + 1) * HD])

                first_chain_done = False
                for h in range(HPC):
                    tils = {}
                    for t in ("q", "k"):
                        ps = qkvps.tile([128, SC_W], fp32, tag="qkv",
                                        name=f"ps_{t}")
                        for dt in range(DT):
                            nc.tensor.matmul(
                                ps[:],
                                w_sb[t][:, dt, h * HD:(h + 1) * HD],
                                xts[:, dt, :],
                                start=(dt == 0), stop=(dt == DT - 1))
                        if not first_chain_done:
                            first_chain_done = True
                            if sc >= 1 and (sc - 1) in pending:
                                # normalize previous chunk while this chunk's
                                # q-chain covers the PE
                                normalize_chunk(sc - 1)
                        til = spool.tile([128, SC_W], bf16, tag="til",
                                         name=f"til_{t}")
                        if t == "q":
                            nc.scalar.activation(til[:], ps[:], Copy,
                                                 scale=SCALE)
                        else:
                            nc.scalar.copy(til[:], ps[:])
                        tils[t] = til
                        if t == "k":
                            emit_rope_mm("q", h, tils["q"])
                    emit_rope_mm("k", h, tils["k"])
                    if h == HPC - 1 and sc == 0:
                        # chunk 0's V runs inline after the Q/K chains (wv
                        # lands on the scalar ring by then); later chunks'
                        # V is woven into the previous chunk's attention
                        for _ in v_gen(0, xts):
                            pass

                # prefetch next chunk's x tiles (sync ring, behind chunk 0's
                # slabs / ahead of the ag reads woven below)
                if sc + 1 < NSC:
                    nxt = xpool.tile([128, DT, SC_W], bf16, tag="xt",
                                     name=f"xt{sc + 1}")
                    nc.sync.dma_start(nxt[:], xt_d[:, sc + 1, :, :])
                    carried_xts = nxt
                else:
                    carried_xts = None

                # weave next chunk's V and a ready chunk's O-proj through
                # this chunk's attention (exp-bound) section
                if carried_xts is not None:
                    op_queue.append(v_gen(sc + 1, carried_xts))
                if sc == LAST:
                    # prefetch oproj(0)+(1)'s reads while QKV(7) runs and
                    # weave their chains into attn(7)
                    agl0 = agl_prefetch(0)
                    agl1 = agl_prefetch(1)
                    op_queue.append(oproj_gen(0, agl=agl0))
                    op_queue.append(oproj_gen(1, agl=agl1))
                # the last two chunks keep running probs-sums so their
                # normalizes can be emitted right at their attention's end:
                # AG(6) then cannot queue-collide with the tail's AG(7)
                attn_chunk(qg, b, inc_sum=(sc >= LAST - 1))
                if sc == LAST - 1:
                    normalize_chunk(sc)
                flush_ops()

            # ============ tail ============
            # AG(7)'s ~20us mesh cycle is covered by oproj(4,5,6); reads
            # are staged so the 4-buffer agl pool never cycles onto a tile
            # whose readers aren't emitted yet. A short dependency-free
            # chain keeps the PE clock ramped across any residual AG wait
            # so oproj(7) runs warm.
            normalize_chunk(LAST)
            agl2 = agl_prefetch(2)
            agl3 = agl_prefetch(3)
            oproj_chunk(2, agl=agl2)
            agl4 = agl_prefetch(4)
            oproj_chunk(3, agl=agl3)
            agl5 = agl_prefetch(5)
            oproj_chunk(4, agl=agl4)
            agl6 = agl_prefetch(6)
            oproj_chunk(5, agl=agl5)
            oproj_chunk(6, agl=agl6)
            # the last chunk's reads + writes ride the scalar ring: the sync
            # ring's FIFO head is parked on the AG(7)-done wait by now
            agl7 = []
            for i in range(4):
                t = aglpool.tile([128, 4, SC_W], bf16, tag="agl",
                                 name=f"agl7_{i}")
                nc.scalar.dma_start(
                    t[:], ag_out_d[LAST][i * 4:(i + 1) * 4, :, :]
                    .rearrange("t p w -> p t w"))
                agl7.append(t)
            # short bridge only: the exec metric is set by the slowest core,
            # which sees AG(7) complete before its o-proj cover runs out —
            # a long dummy here would be pure delay on that core
            dmy_ps = opps.tile([128, SC_W], fp32, tag="op", name="dmy_ps")
            for i in range(8):
                nc.tensor.matmul(dmy_ps[:], on2_sb[:], cos_sb[:, 0:SC_W],
                                 start=(i == 0), stop=(i == 7))
            dmy_sb = obpool.tile([128, SC_W], bf16, tag="ob", name="dmy_sb")
            nc.scalar.copy(dmy_sb[:], dmy_ps[:])
            nc.sync.dma_start(warm_in_d[0:8, 0:16], dmy_sb[0:8, 0:16])
            for _ in oproj_emit(LAST, range(DT),
                                lambda e: agl7[e // 4][:, e % 4, :]):
                pass

    nc.compile()
    return nc


def _get_nc(S):
    if S not in _CACHE:
        _CACHE[S] = _build(S)
    return _CACHE[S]


def make_inputs(x, freqs_cis, mask, wq, wk, wv, wo):
    """Host-side sharding / layout prep. Returns in_maps for 8 cores."""
    S = x.shape[1]
    nsc = (B * S) // SC_W
    flat_xt = np.ascontiguousarray(np.asarray(x, np.float32).reshape(B * S, DIM).T)
    # xt partition-contiguous: [128, NSC, DT, SC_W]
    xt_pc = np.ascontiguousarray(
        flat_xt.reshape(DT, 128, nsc, SC_W).transpose(1, 2, 0, 3))
    cos = np.asarray(freqs_cis[..., 0], np.float32)   # [S, HD/2]
    sin = np.asarray(freqs_cis[..., 1], np.float32)
    cos_t = np.ascontiguousarray(np.repeat(cos.T, 2, axis=0))  # [HD, S]
    sin_t = np.ascontiguousarray(np.repeat(sin.T, 2, axis=0))
    m = np.asarray(mask, np.float32)[0, 0]
    nqt = S // 128
    mask_diag = np.stack([m[i * 128:(i + 1) * 128, i * 128:(i + 1) * 128].T
                          for i in range(nqt)])          # [NQT, 128, 128]
    mask_pc = np.ascontiguousarray(mask_diag.transpose(1, 0, 2))  # [128,NQT,128]
    import ml_dtypes
    bf = ml_dtypes.bfloat16
    xt_pc = xt_pc.astype(bf)
    cos_t = cos_t.astype(bf)
    sin_t = sin_t.astype(bf)
    # sign of the rotation folded into the sin table (even rows negated)
    sin_t = sin_t.copy()
    sin_t[0::2, :] = -sin_t[0::2, :]

    def w_pc(w_slice):
        # [DIM_in, C] -> [128, DT, C] partition-contiguous
        c = w_slice.shape[1]
        return np.ascontiguousarray(
            w_slice.reshape(DT, 128, c).transpose(1, 0, 2)).astype(bf)

    wq = np.asarray(wq, np.float32)
    wk = np.asarray(wk, np.float32)
    wv = np.asarray(wv, np.float32)
    wo = np.asarray(wo, np.float32)
    in_maps = []
    for c in range(N_CORES):
        r = slice(c * OC, (c + 1) * OC)
        rd = slice(c * OCD, (c + 1) * OCD)
        in_maps.append({
            "xt": xt_pc,
            "wqt": w_pc(np.ascontiguousarray(wq[r, :].T)),
            "wkt": w_pc(np.ascontiguousarray(wk[r, :].T)),
            "wvt": w_pc(np.ascontiguousarray(wv[r, :].T)),
            "wotc": w_pc(np.ascontiguousarray(wo[rd, :].T)),
            "cos_t": cos_t,
            "sin_t": sin_t,
            "mask_diag": mask_pc.astype(bf),
            "ones128": np.ones((128, 128), dtype=bf),
        })
    return in_maps


def assemble(results, S):
    """Column-concat per-core output shards into the full output."""
    full = np.empty((B * S, DIM), np.float32)
    for c in range(N_CORES):
        full[:, c * OCD:(c + 1) * OCD] = \
            np.asarray(results[c]["outT"], np.float32).T
    return full.reshape(B, S, DIM)


def kernel(x, start_pos, freqs_cis, mask, wq, wk, wv, wo):
    from concourse.bass_utils import run_bass_kernel_spmd
    S = x.shape[1]
    nc = _get_nc(S)
    in_maps = make_inputs(x, freqs_cis, mask, wq, wk, wv, wo)
    res = run_bass_kernel_spmd(nc, in_maps, core_ids=list(range(N_CORES)))
    return assemble(res.results, S)
